# revision 6
# baseline (speedup 1.0000x reference)
"""Trainium2 Bass kernel for nn_AbstractAttention (dense transformer attention
with RoPE, B=2 S=2048 D=4096 H=32), tensor-parallel over heads on 8 cores.

Self-contained: builds the Bass program, shards inputs on host, runs via
run_bass_kernel_spmd, and reduces partial outputs on host.
"""

import math

import numpy as np

import concourse.bacc as bacc
import concourse.mybir as mybir
from concourse.bass_utils import run_bass_kernel_spmd
from concourse.masks import make_identity
from concourse.tile import TileContext

P = 128  # partitions / head_dim
F32 = mybir.dt.float32

# set by test.py for profiling; grading path leaves these alone
TRACE = False
TRACE_DIR = None
LAST_RESULT = [None]

ZERO, MIXED, SKIP = 0, 1, 2


def _classify_blocks(maskT, S, QT):
    """maskT: [S, S] (k, q). Returns (kinds[nqt][nkc], mixed_tiles, mixed_index)."""
    nqt, nkc = S // QT, S // P
    kinds = [[ZERO] * nkc for _ in range(nqt)]
    tiles = []
    index = {}
    for qt in range(nqt):
        for kc in range(nkc):
            sub = maskT[kc * P : (kc + 1) * P, qt * QT : (qt + 1) * QT]
            if np.all(sub == 0.0):
                kinds[qt][kc] = ZERO
            elif np.all(np.isneginf(sub) | (sub < -1e30)):
                kinds[qt][kc] = SKIP
            else:
                kinds[qt][kc] = MIXED
                index[(qt, kc)] = len(tiles)
                tiles.append(np.maximum(sub, -1e30))
    if tiles:
        mixed = np.ascontiguousarray(np.stack(tiles)).astype(np.float32)
    else:
        mixed = np.zeros((1, P, QT), dtype=np.float32)
    return kinds, mixed, index


def _build(B, S, D, HL, kinds, mixed_index, n_mixed):
    """Build the per-core Bass program.

    HL: local heads on this core. Local width DL = HL*128.
    Inputs: xT [D, T], wqT/wkT/wvT [D, DL], woT [DL, D], cos_e/nsin_e [S, P],
            mask_tiles [n_mixed, P, QT].  Output: out [T, D] (partial).
    """
    DL = HL * P
    T = B * S
    NCH = T // P  # token chunks
    CHB = S // P  # chunks per batch
    QT = 512
    NQT = S // QT
    NKC = S // P
    KD = D // P  # contraction subtiles for projections
    inv_sqrt_d = 1.0 / math.sqrt(P)

    nc = bacc.Bacc(None, target_bir_lowering=False)

    xT = nc.declare_dram_parameter("xT", [D, T], F32, isOutput=False)
    wqT = nc.declare_dram_parameter("wqT", [D, DL], F32, isOutput=False)
    wkT = nc.declare_dram_parameter("wkT", [D, DL], F32, isOutput=False)
    wvT = nc.declare_dram_parameter("wvT", [D, DL], F32, isOutput=False)
    woT = nc.declare_dram_parameter("woT", [DL, D], F32, isOutput=False)
    cos_e = nc.declare_dram_parameter("cos_e", [S, P], F32, isOutput=False)
    nsin_e = nc.declare_dram_parameter("nsin_e", [S, P], F32, isOutput=False)
    mtiles = nc.declare_dram_parameter(
        "mask_tiles", [max(n_mixed, 1), P, QT], F32, isOutput=False
    )
    out = nc.declare_dram_parameter("out", [T, D], F32, isOutput=True)

    ts = lambda i, s: slice(i * s, (i + 1) * s)

    with TileContext(nc) as tc:
        with (
            tc.tile_pool(name="consts", bufs=1) as consts,
            tc.tile_pool(name="wpool", bufs=1) as wpool,
            tc.tile_pool(name="xpool", bufs=3) as xpool,
            tc.tile_pool(name="cpool", bufs=2) as cpool,
            tc.tile_pool(name="qtp", bufs=2) as qtp,
            tc.tile_pool(name="work", bufs=2) as work,
            tc.tile_pool(name="stage", bufs=2) as stage,
            tc.tile_pool(name="a2", bufs=1) as a2,
            tc.tile_pool(name="small", bufs=4) as small,
            tc.tile_pool(name="psA", bufs=2, space="PSUM") as psA,
            tc.tile_pool(name="psCtx", bufs=1, space="PSUM") as psCtx,
            tc.tile_pool(name="psT", bufs=2, space="PSUM") as psT,
            tc.tile_pool(name="dram", bufs=1, space="DRAM") as dram,
        ):
            ident = consts.tile([P, P], F32)
            make_identity(nc, ident)
            ones_h = consts.tile([P, HL], F32)
            nc.vector.memset(ones_h, 1.0)
            cos_sb = consts.tile([P, CHB, P], F32)
            nsin_sb = consts.tile([P, CHB, P], F32)
            nc.sync.dma_start(cos_sb, cos_e.ap().rearrange("(o p) d -> p o d", p=P))
            nc.sync.dma_start(nsin_sb, nsin_e.ap().rearrange("(o p) d -> p o d", p=P))

            # mixed-mask tiles kept SBUF-resident (small for causal masks)
            mcache = {}
            cache_mask = n_mixed <= 8
            if cache_mask:
                for (qt, kc), idx in mixed_index.items():
                    mt = consts.tile([P, QT], F32, tag=f"m{qt}_{kc}", name=f"m{qt}_{kc}")
                    nc.sync.dma_start(mt, mtiles.ap()[idx])
                    mcache[(qt, kc)] = mt

            # DRAM scratch
            qT_scr = [
                [dram.tile([P, S], F32, tag=f"qT{b}_{h}", name=f"qT{b}_{h}") for h in range(HL)]
                for b in range(B)
            ]
            kT_scr = [
                [dram.tile([P, S], F32, tag=f"kT{b}_{h}", name=f"kT{b}_{h}") for h in range(HL)]
                for b in range(B)
            ]
            v_scr = [dram.tile([S, HL, P + 1], F32, tag=f"v{b}", name=f"v{b}") for b in range(B)]
            ctxT_scr = [dram.tile([DL, S], F32, tag=f"ctxT{b}", name=f"ctxT{b}") for b in range(B)]

            # ones columns of v_scr (softmax denominator trick)
            for b in range(B):
                for c in range(CHB):
                    nc.sync.dma_start(
                        v_scr[b][ts(c, P), :, P : P + 1], ones_h[:, :, None]
                    )

            xT_r = xT.ap().rearrange("(o p) t -> p o t", p=P)

            # ---------------- Phase 1: projections (+rope for q/k) -------------
            def proj_pass(wT, rope, name):
                w_sb = wpool.tile([P, KD, DL], F32, tag="w")
                nc.sync.dma_start(w_sb, wT.ap().rearrange("(o p) n -> p o n", p=P))
                KH = KD // 2
                for ch in range(NCH):
                    b, so = ch // CHB, ch % CHB
                    ps = psA.tile([P, DL], F32, tag="ps_a")
                    for half in range(2):
                        x_sb = xpool.tile([P, KH, P], F32, tag="xT")
                        nc.sync.dma_start(
                            x_sb, xT_r[:, ts(half, KH), ts(ch, P)]
                        )
                        for j in range(KH):
                            nc.tensor.matmul(
                                ps,
                                x_sb[:, j, :],
                                w_sb[:, half * KH + j, :],
                                start=(half == 0 and j == 0),
                                stop=(half == 1 and j == KH - 1),
                            )
                    if rope:
                        cosv = cos_sb[:, so, None, :].to_broadcast((P, HL, P))
                        t1 = work.tile([P, DL], F32, tag="t1")
                        t1v = t1.rearrange("p (h d) -> p h d", h=HL)
                        psv = ps.rearrange("p (h d) -> p h d", h=HL)
                        nc.vector.tensor_tensor(t1v, psv, cosv, mybir.AluOpType.mult)
                        t2 = work.tile([P, DL], F32, tag="t2")
                        t2v = t2.rearrange("p (h d) -> p h d", h=HL)
                        ps4 = ps.rearrange("p (h n two) -> p h n two", h=HL, two=2)
                        t24 = t2.rearrange("p (h n two) -> p h n two", h=HL, two=2)
                        nsv = nsin_sb[:, so, :].rearrange("p (n two) -> p n two", two=2)
                        nc.vector.tensor_tensor(
                            t24[:, :, :, 0:1],
                            ps4[:, :, :, 1:2],
                            nsv[:, None, :, 0:1].to_broadcast((P, HL, P // 2, 1)),
                            mybir.AluOpType.mult,
                        )
                        nc.vector.tensor_tensor(
                            t24[:, :, :, 1:2],
                            ps4[:, :, :, 0:1],
                            nsv[:, None, :, 1:2].to_broadcast((P, HL, P // 2, 1)),
                            mybir.AluOpType.mult,
                        )
                        qf = work.tile([P, DL], F32, tag="qf")
                        nc.vector.tensor_tensor(qf, t1, t2, mybir.AluOpType.add)
                        qfv = qf.rearrange("p (h d) -> p h d", h=HL)
                        scr = qT_scr if name == "q" else kT_scr
                        for h in range(HL):
                            pt = psT.tile([P, P], F32, tag="ptr")
                            nc.tensor.transpose(pt, qfv[:, h, :], ident)
                            st = stage.tile([P, P], F32, tag="trs")
                            nc.scalar.copy(st, pt)
                            nc.sync.dma_start(scr[b][h][:, ts(so, P)], st)
                    else:
                        st = stage.tile([P, DL], F32, tag="vst")
                        nc.scalar.copy(st, ps)
                        nc.sync.dma_start(
                            v_scr[b][ts(so, P), :, 0:P],
                            st.rearrange("p (h d) -> p h d", h=HL),
                        )

            proj_pass(wqT, True, "q")
            proj_pass(wkT, True, "k")
            proj_pass(wvT, False, "v")

            # ---------------- Phase 2: attention -----------------------------
            active = [
                [kc for kc in range(NKC) if kinds[qt][kc] != SKIP]
                for qt in range(NQT)
            ]
            for b in range(B):
                for h in range(HL):
                    kT_sb = a2.tile([P, S], F32, tag="kT_sb")
                    nc.sync.dma_start(kT_sb, kT_scr[b][h][:])
                    v_sb = a2.tile([P, NKC, P + 1], F32, tag="v_sb")
                    nc.sync.dma_start(
                        v_sb,
                        v_scr[b][:, h, :].rearrange("(o p) c -> p o c", p=P),
                    )
                    for qt in range(NQT):
                        acts = active[qt]
                        if not acts:
                            continue
                        qT_sb = qtp.tile([P, QT], F32, tag="qT_t")
                        nc.sync.dma_start(qT_sb, qT_scr[b][h][:, ts(qt, QT)])
                        ctx_ps = [
                            psCtx.tile([P, P + 1], F32, tag=f"ctx{i}", name=f"ctx{i}")
                            for i in range(QT // P)
                        ]
                        for ki, kc in enumerate(acts):
                            sps = psA.tile([P, QT], F32, tag="ps_a")
                            nc.tensor.matmul(
                                sps,
                                kT_sb[:, ts(kc, P)],
                                qT_sb[:],
                                start=True,
                                stop=True,
                            )
                            pT = work.tile([P, QT], F32, tag="pT")
                            if kinds[qt][kc] == MIXED:
                                if cache_mask:
                                    mt = mcache[(qt, kc)]
                                else:
                                    mt = work.tile([P, QT], F32, tag="mload")
                                    nc.sync.dma_start(
                                        mt, mtiles.ap()[mixed_index[(qt, kc)]]
                                    )
                                nc.vector.scalar_tensor_tensor(
                                    sps,
                                    sps,
                                    inv_sqrt_d,
                                    mt,
                                    mybir.AluOpType.mult,
                                    mybir.AluOpType.add,
                                )
                                nc.scalar.activation(
                                    pT, sps, mybir.ActivationFunctionType.Exp
                                )
                            else:
                                nc.scalar.activation(
                                    pT,
                                    sps,
                                    mybir.ActivationFunctionType.Exp,
                                    scale=inv_sqrt_d,
                                )
                            for qs in range(QT // P):
                                nc.tensor.matmul(
                                    ctx_ps[qs],
                                    pT[:, ts(qs, P)],
                                    v_sb[:, kc, :],
                                    start=(ki == 0),
                                    stop=(ki == len(acts) - 1),
                                )
                        for qs in range(QT // P):
                            rec = small.tile([P, 1], F32, tag="rec")
                            nc.vector.reciprocal(rec, ctx_ps[qs][:, P : P + 1])
                            cn = stage.tile([P, P], F32, tag="cn")
                            nc.vector.tensor_scalar_mul(cn, ctx_ps[qs][:, 0:P], rec)
                            pt = psT.tile([P, P], F32, tag="ptr")
                            nc.tensor.transpose(pt, cn, ident)
                            ct = stage.tile([P, P], F32, tag="cts")
                            nc.scalar.copy(ct, pt)
                            nc.sync.dma_start(
                                ctxT_scr[b][
                                    ts(h, P), ts(qt * (QT // P) + qs, P)
                                ],
                                ct,
                            )

            # ---------------- Phase 3: output projection ----------------------
            wo_sb = wpool.tile([P, HL, D], F32, tag="w")
            nc.sync.dma_start(wo_sb, woT.ap().rearrange("(o p) n -> p o n", p=P))
            for b in range(B):
                ctxr = ctxT_scr[b].rearrange("(o p) t -> p o t", p=P)
                for c in range(CHB):
                    cT = cpool.tile([P, HL, P], F32, tag="cT")
                    nc.sync.dma_start(cT, ctxr[:, :, ts(c, P)])
                    for oc in range(D // 512):
                        ps = psA.tile([P, 512], F32, tag="ps_a")
                        for j in range(HL):
                            nc.tensor.matmul(
                                ps,
                                cT[:, j, :],
                                wo_sb[:, j, ts(oc, 512)],
                                start=(j == 0),
                                stop=(j == HL - 1),
                            )
                        st = stage.tile([P, 512], F32, tag="ost")
                        nc.scalar.copy(st, ps)
                        nc.sync.dma_start(
                            out.ap()[ts(b * CHB + c, P), ts(oc, 512)], st
                        )

    nc.finalize()
    return nc


def kernel(x, wq, wk, wv, wo, cos, sin, mask):
    B, S, D = x.shape
    H = D // P
    NCORES = 8
    HL = H // NCORES
    DL = HL * P
    T = B * S

    x = np.asarray(x, dtype=np.float32)
    xT = np.ascontiguousarray(x.reshape(T, D).T)
    cos = np.asarray(cos, dtype=np.float32)
    sin = np.asarray(sin, dtype=np.float32)
    cos_e = np.repeat(cos, 2, axis=1).astype(np.float32)  # [S, 128]
    nsin_e = np.empty((S, P), dtype=np.float32)
    nsin_e[:, 0::2] = -sin
    nsin_e[:, 1::2] = sin

    maskT = np.ascontiguousarray(np.asarray(mask, dtype=np.float32)[0, 0].T)
    kinds, mixed, mixed_index = _classify_blocks(maskT, S, 512)

    nc = _build(B, S, D, HL, kinds, mixed_index, len(mixed))

    wq = np.asarray(wq, dtype=np.float32)
    wk = np.asarray(wk, dtype=np.float32)
    wv = np.asarray(wv, dtype=np.float32)
    wo = np.asarray(wo, dtype=np.float32)

    in_maps = []
    for c in range(NCORES):
        sl = slice(c * DL, (c + 1) * DL)
        in_maps.append(
            {
                "xT": xT,
                "wqT": np.ascontiguousarray(wq[sl, :].T),
                "wkT": np.ascontiguousarray(wk[sl, :].T),
                "wvT": np.ascontiguousarray(wv[sl, :].T),
                "woT": np.ascontiguousarray(wo[:, sl].T),
                "cos_e": cos_e,
                "nsin_e": nsin_e,
                "mask_tiles": mixed,
            }
        )

    kwargs = {}
    if TRACE:
        kwargs = {"trace": True}
        if TRACE_DIR:
            kwargs["tmpdir"] = TRACE_DIR
    res = run_bass_kernel_spmd(nc, in_maps, core_ids=list(range(NCORES)), **kwargs)
    LAST_RESULT[0] = res

    acc = res.results[0]["out"].astype(np.float64)
    for c in range(1, NCORES):
        acc += res.results[c]["out"]
    return acc.astype(np.float32).reshape(B, S, D)


# revision 8
# speedup vs baseline: 1.0077x; 1.0077x over previous
"""Trainium2 Bass kernel for nn_AbstractAttention (dense transformer attention
with RoPE, B=2 S=2048 D=4096 H=32), tensor-parallel over heads on 8 cores.

Self-contained: builds the Bass program, shards inputs on host, runs via
run_bass_kernel_spmd, and reduces partial outputs on host.
"""

import math

import numpy as np

import concourse.bacc as bacc
import concourse.mybir as mybir
from concourse.bass_utils import run_bass_kernel_spmd
from concourse.masks import make_identity
from concourse.tile import TileContext

P = 128  # partitions / head_dim
F32 = mybir.dt.float32

# set by test.py for profiling; grading path leaves these alone
TRACE = False
TRACE_DIR = None
LAST_RESULT = [None]

ZERO, MIXED, SKIP = 0, 1, 2


def _tf32(a):
    u = np.ascontiguousarray(a, dtype=np.float32).view(np.uint32)
    r = ((u >> 13).astype(np.uint64) + ((u >> 12) & 1)) << 13
    return (r & 0xFFFFFFFF).astype(np.uint32).view(np.float32)


def _classify_blocks(maskT, S, QT):
    """maskT: [S, S] (k, q). Returns (kinds[nqt][nkc], mixed_tiles, mixed_index)."""
    nqt, nkc = S // QT, S // P
    kinds = [[ZERO] * nkc for _ in range(nqt)]
    tiles = []
    index = {}
    for qt in range(nqt):
        for kc in range(nkc):
            sub = maskT[kc * P : (kc + 1) * P, qt * QT : (qt + 1) * QT]
            if np.all(sub == 0.0):
                kinds[qt][kc] = ZERO
            elif np.all(np.isneginf(sub) | (sub < -1e30)):
                kinds[qt][kc] = SKIP
            else:
                kinds[qt][kc] = MIXED
                index[(qt, kc)] = len(tiles)
                tiles.append(np.maximum(sub, -1e30))
    if tiles:
        mixed = np.ascontiguousarray(np.stack(tiles)).astype(np.float32)
    else:
        mixed = np.zeros((1, P, QT), dtype=np.float32)
    return kinds, mixed, index


def _build(B, S, D, HL, kinds, mixed_index, n_mixed):
    """Build the per-core Bass program.

    HL: local heads on this core. Local width DL = HL*128.
    Inputs: xT [D, T], wqT/wkT/wvT [D, DL], woT [DL, D], cos_e/nsin_e [S, P],
            mask_tiles [n_mixed, P, QT].  Output: out [T, D] (partial).
    """
    DL = HL * P
    T = B * S
    NCH = T // P  # token chunks
    CHB = S // P  # chunks per batch
    QT = 512
    NQT = S // QT
    NKC = S // P
    KD = D // P  # contraction subtiles for projections
    inv_sqrt_d = 1.0 / math.sqrt(P)

    nc = bacc.Bacc(None, target_bir_lowering=False)

    F32R = mybir.dt.float32r
    xTh = nc.declare_dram_parameter("xTh", [D, T], F32R, isOutput=False)
    xTl = nc.declare_dram_parameter("xTl", [D, T], F32R, isOutput=False)
    wqTh = nc.declare_dram_parameter("wqTh", [D, DL], F32R, isOutput=False)
    wqTl = nc.declare_dram_parameter("wqTl", [D, DL], F32R, isOutput=False)
    wkTh = nc.declare_dram_parameter("wkTh", [D, DL], F32R, isOutput=False)
    wkTl = nc.declare_dram_parameter("wkTl", [D, DL], F32R, isOutput=False)
    wvTh = nc.declare_dram_parameter("wvTh", [D, DL], F32R, isOutput=False)
    wvTl = nc.declare_dram_parameter("wvTl", [D, DL], F32R, isOutput=False)
    woT = nc.declare_dram_parameter("woT", [DL, D], F32, isOutput=False)
    cos_e = nc.declare_dram_parameter("cos_e", [S, P], F32, isOutput=False)
    nsin_e = nc.declare_dram_parameter("nsin_e", [S, P], F32, isOutput=False)
    mtiles = nc.declare_dram_parameter(
        "mask_tiles", [max(n_mixed, 1), P, QT], F32, isOutput=False
    )
    out = nc.declare_dram_parameter("out", [T, D], F32, isOutput=True)

    ts = lambda i, s: slice(i * s, (i + 1) * s)

    with TileContext(nc) as tc:
        with (
            tc.tile_pool(name="consts", bufs=1) as consts,
            tc.tile_pool(name="wpool", bufs=1) as wpool,
            tc.tile_pool(name="xpool", bufs=3) as xpool,
            tc.tile_pool(name="cpool", bufs=2) as cpool,
            tc.tile_pool(name="qtp", bufs=2) as qtp,
            tc.tile_pool(name="work", bufs=2) as work,
            tc.tile_pool(name="stage", bufs=2) as stage,
            tc.tile_pool(name="a2", bufs=1) as a2,
            tc.tile_pool(name="small", bufs=4) as small,
            tc.tile_pool(name="psA", bufs=2, space="PSUM") as psA,
            tc.tile_pool(name="psCtx", bufs=1, space="PSUM") as psCtx,
            tc.tile_pool(name="psT", bufs=2, space="PSUM") as psT,
            tc.tile_pool(name="dram", bufs=1, space="DRAM") as dram,
        ):
            ident = consts.tile([P, P], F32)
            make_identity(nc, ident)
            ones_h = consts.tile([P, HL], F32)
            nc.vector.memset(ones_h, 1.0)
            cos_sb = consts.tile([P, CHB, P], F32)
            nsin_sb = consts.tile([P, CHB, P], F32)
            nc.sync.dma_start(cos_sb, cos_e.ap().rearrange("(o p) d -> p o d", p=P))
            nc.sync.dma_start(nsin_sb, nsin_e.ap().rearrange("(o p) d -> p o d", p=P))

            # mixed-mask tiles kept SBUF-resident (small for causal masks)
            mcache = {}
            cache_mask = n_mixed <= 8
            if cache_mask:
                for (qt, kc), idx in mixed_index.items():
                    mt = consts.tile([P, QT], F32, tag=f"m{qt}_{kc}", name=f"m{qt}_{kc}")
                    nc.sync.dma_start(mt, mtiles.ap()[idx])
                    mcache[(qt, kc)] = mt

            # DRAM scratch
            qT_scr = [
                [dram.tile([P, S], F32, tag=f"qT{b}_{h}", name=f"qT{b}_{h}") for h in range(HL)]
                for b in range(B)
            ]
            kT_scr = [
                [dram.tile([P, S], F32, tag=f"kT{b}_{h}", name=f"kT{b}_{h}") for h in range(HL)]
                for b in range(B)
            ]
            v_scr = [dram.tile([S, HL, P + 1], F32, tag=f"v{b}", name=f"v{b}") for b in range(B)]
            ctxT_scr = [dram.tile([DL, S], F32, tag=f"ctxT{b}", name=f"ctxT{b}") for b in range(B)]

            # ones columns of v_scr (softmax denominator trick)
            for b in range(B):
                for c in range(CHB):
                    nc.sync.dma_start(
                        v_scr[b][ts(c, P), :, P : P + 1], ones_h[:, :, None]
                    )

            xTh_r = xTh.ap().rearrange("(o p) t -> p o t", p=P)
            xTl_r = xTl.ap().rearrange("(o p) t -> p o t", p=P)
            pp = dram.tile([T, DL], F32, tag="pp", name="pp")

            # ---------------- Phase 1: projections (+rope for q/k) -------------
            # 3-term tf32 split: x@w ~= xh@wh + xh@wl + xl@wh, fp32r matmuls.
            def proj_pass(wTh, wTl, rope, name):
                KH = KD // 2
                for half in range(2):
                    w_h = wpool.tile([P, KH, DL], F32R, tag="wh")
                    nc.sync.dma_start(
                        w_h,
                        wTh.ap().rearrange("(o p) n -> p o n", p=P)[:, ts(half, KH)],
                    )
                    w_l = wpool.tile([P, KH, DL], F32R, tag="wl")
                    nc.sync.dma_start(
                        w_l,
                        wTl.ap().rearrange("(o p) n -> p o n", p=P)[:, ts(half, KH)],
                    )
                    for ch in range(NCH):
                        b, so = ch // CHB, ch % CHB
                        x_h = xpool.tile([P, KH, P], F32R, tag="xh")
                        nc.sync.dma_start(x_h, xTh_r[:, ts(half, KH), ts(ch, P)])
                        x_l = xpool.tile([P, KH, P], F32R, tag="xl")
                        nc.sync.dma_start(x_l, xTl_r[:, ts(half, KH), ts(ch, P)])
                        ps = psA.tile([P, DL], F32, tag="ps_a")
                        i = 0
                        nterm = 3 * KH
                        for a, w in [(x_h, w_h), (x_h, w_l), (x_l, w_h)]:
                            for j in range(KH):
                                nc.tensor.matmul(
                                    ps,
                                    a[:, j, :],
                                    w[:, j, :],
                                    start=(i == 0),
                                    stop=(i == nterm - 1),
                                )
                                i += 1
                        if half == 0:
                            pst = stage.tile([P, DL], F32, tag="pst")
                            nc.scalar.copy(pst, ps)
                            nc.sync.dma_start(pp[ts(ch, P), :], pst)
                            continue
                        pa = stage.tile([P, DL], F32, tag="pa")
                        nc.sync.dma_start(pa, pp[ts(ch, P), :])
                        if not rope:
                            vst = stage.tile([P, DL], F32, tag="vst")
                            nc.vector.tensor_tensor(
                                vst, ps, pa, mybir.AluOpType.add
                            )
                            nc.sync.dma_start(
                                v_scr[b][ts(so, P), :, 0:P],
                                vst.rearrange("p (h d) -> p h d", h=HL),
                            )
                            continue
                        qs = work.tile([P, DL], F32, tag="qs")
                        nc.vector.tensor_tensor(qs, ps, pa, mybir.AluOpType.add)
                        cosv = cos_sb[:, so, None, :].to_broadcast((P, HL, P))
                        t1 = work.tile([P, DL], F32, tag="t1")
                        t1v = t1.rearrange("p (h d) -> p h d", h=HL)
                        qsv = qs.rearrange("p (h d) -> p h d", h=HL)
                        nc.vector.tensor_tensor(t1v, qsv, cosv, mybir.AluOpType.mult)
                        t2 = work.tile([P, DL], F32, tag="t2")
                        qs4 = qs.rearrange("p (h n two) -> p h n two", h=HL, two=2)
                        t24 = t2.rearrange("p (h n two) -> p h n two", h=HL, two=2)
                        nsv = nsin_sb[:, so, :].rearrange("p (n two) -> p n two", two=2)
                        nc.vector.tensor_tensor(
                            t24[:, :, :, 0:1],
                            qs4[:, :, :, 1:2],
                            nsv[:, None, :, 0:1].to_broadcast((P, HL, P // 2, 1)),
                            mybir.AluOpType.mult,
                        )
                        nc.vector.tensor_tensor(
                            t24[:, :, :, 1:2],
                            qs4[:, :, :, 0:1],
                            nsv[:, None, :, 1:2].to_broadcast((P, HL, P // 2, 1)),
                            mybir.AluOpType.mult,
                        )
                        qf = work.tile([P, DL], F32, tag="qf")
                        nc.vector.tensor_tensor(qf, t1, t2, mybir.AluOpType.add)
                        qfv = qf.rearrange("p (h d) -> p h d", h=HL)
                        scr = qT_scr if name == "q" else kT_scr
                        for h in range(HL):
                            pt = psT.tile([P, P], F32, tag="ptr")
                            nc.tensor.transpose(pt, qfv[:, h, :], ident)
                            st = stage.tile([P, P], F32, tag="trs")
                            nc.scalar.copy(st, pt)
                            nc.sync.dma_start(scr[b][h][:, ts(so, P)], st)

            proj_pass(wqTh, wqTl, True, "q")
            proj_pass(wkTh, wkTl, True, "k")
            proj_pass(wvTh, wvTl, False, "v")

            # ---------------- Phase 2: attention -----------------------------
            active = [
                [kc for kc in range(NKC) if kinds[qt][kc] != SKIP]
                for qt in range(NQT)
            ]
            for b in range(B):
                for h in range(HL):
                    kT_sb = a2.tile([P, S], F32, tag="kT_sb")
                    nc.sync.dma_start(kT_sb, kT_scr[b][h][:])
                    v_sb = a2.tile([P, NKC, P + 1], F32, tag="v_sb")
                    nc.sync.dma_start(
                        v_sb,
                        v_scr[b][:, h, :].rearrange("(o p) c -> p o c", p=P),
                    )
                    for qt in range(NQT):
                        acts = active[qt]
                        if not acts:
                            continue
                        qT_sb = qtp.tile([P, QT], F32, tag="qT_t")
                        nc.sync.dma_start(qT_sb, qT_scr[b][h][:, ts(qt, QT)])
                        ctx_ps = [
                            psCtx.tile([P, P + 1], F32, tag=f"ctx{i}", name=f"ctx{i}")
                            for i in range(QT // P)
                        ]
                        for ki, kc in enumerate(acts):
                            sps = psA.tile([P, QT], F32, tag="ps_a")
                            nc.tensor.matmul(
                                sps,
                                kT_sb[:, ts(kc, P)],
                                qT_sb[:],
                                start=True,
                                stop=True,
                            )
                            pT = work.tile([P, QT], F32, tag="pT")
                            if kinds[qt][kc] == MIXED:
                                if cache_mask:
                                    mt = mcache[(qt, kc)]
                                else:
                                    mt = work.tile([P, QT], F32, tag="mload")
                                    nc.sync.dma_start(
                                        mt, mtiles.ap()[mixed_index[(qt, kc)]]
                                    )
                                nc.vector.scalar_tensor_tensor(
                                    sps,
                                    sps,
                                    inv_sqrt_d,
                                    mt,
                                    mybir.AluOpType.mult,
                                    mybir.AluOpType.add,
                                )
                                nc.scalar.activation(
                                    pT, sps, mybir.ActivationFunctionType.Exp
                                )
                            else:
                                nc.scalar.activation(
                                    pT,
                                    sps,
                                    mybir.ActivationFunctionType.Exp,
                                    scale=inv_sqrt_d,
                                )
                            for qs in range(QT // P):
                                nc.tensor.matmul(
                                    ctx_ps[qs],
                                    pT[:, ts(qs, P)],
                                    v_sb[:, kc, :],
                                    start=(ki == 0),
                                    stop=(ki == len(acts) - 1),
                                )
                        for qs in range(QT // P):
                            rec = small.tile([P, 1], F32, tag="rec")
                            nc.vector.reciprocal(rec, ctx_ps[qs][:, P : P + 1])
                            cn = stage.tile([P, P], F32, tag="cn")
                            nc.vector.tensor_scalar_mul(cn, ctx_ps[qs][:, 0:P], rec)
                            pt = psT.tile([P, P], F32, tag="ptr")
                            nc.tensor.transpose(pt, cn, ident)
                            ct = stage.tile([P, P], F32, tag="cts")
                            nc.scalar.copy(ct, pt)
                            nc.sync.dma_start(
                                ctxT_scr[b][
                                    ts(h, P), ts(qt * (QT // P) + qs, P)
                                ],
                                ct,
                            )

            # ---------------- Phase 3: output projection ----------------------
            wo_r = woT.ap().rearrange("(o p) n -> p o n", p=P)
            wo_a = wpool.tile([P, HL // 2, D], F32, tag="wh")
            nc.sync.dma_start(wo_a, wo_r[:, : HL // 2])
            wo_b = wpool.tile([P, HL // 2, D], F32, tag="wl")
            nc.sync.dma_start(wo_b, wo_r[:, HL // 2 :])
            for b in range(B):
                ctxr = ctxT_scr[b].rearrange("(o p) t -> p o t", p=P)
                for c in range(CHB):
                    cT = cpool.tile([P, HL, P], F32, tag="cT")
                    nc.sync.dma_start(cT, ctxr[:, :, ts(c, P)])
                    for oc in range(D // 512):
                        ps = psA.tile([P, 512], F32, tag="ps_a")
                        for j in range(HL):
                            wsb = wo_a if j < HL // 2 else wo_b
                            nc.tensor.matmul(
                                ps,
                                cT[:, j, :],
                                wsb[:, j % (HL // 2), ts(oc, 512)],
                                start=(j == 0),
                                stop=(j == HL - 1),
                            )
                        st = stage.tile([P, 512], F32, tag="ost")
                        nc.scalar.copy(st, ps)
                        nc.sync.dma_start(
                            out.ap()[ts(b * CHB + c, P), ts(oc, 512)], st
                        )

    nc.finalize()
    return nc


def kernel(x, wq, wk, wv, wo, cos, sin, mask):
    B, S, D = x.shape
    H = D // P
    NCORES = 8
    HL = H // NCORES
    DL = HL * P
    T = B * S

    x = np.asarray(x, dtype=np.float32)
    xT = np.ascontiguousarray(x.reshape(T, D).T)
    xTh = _tf32(xT)
    xTl = _tf32(xT - xTh)
    cos = np.asarray(cos, dtype=np.float32)
    sin = np.asarray(sin, dtype=np.float32)
    cos_e = np.repeat(cos, 2, axis=1).astype(np.float32)  # [S, 128]
    nsin_e = np.empty((S, P), dtype=np.float32)
    nsin_e[:, 0::2] = -sin
    nsin_e[:, 1::2] = sin

    maskT = np.ascontiguousarray(np.asarray(mask, dtype=np.float32)[0, 0].T)
    kinds, mixed, mixed_index = _classify_blocks(maskT, S, 512)

    nc = _build(B, S, D, HL, kinds, mixed_index, len(mixed))

    wq = np.asarray(wq, dtype=np.float32)
    wk = np.asarray(wk, dtype=np.float32)
    wv = np.asarray(wv, dtype=np.float32)
    wo = np.asarray(wo, dtype=np.float32)

    in_maps = []
    for c in range(NCORES):
        sl = slice(c * DL, (c + 1) * DL)
        m = {
            "xTh": xTh,
            "xTl": xTl,
            "woT": np.ascontiguousarray(wo[:, sl].T),
            "cos_e": cos_e,
            "nsin_e": nsin_e,
            "mask_tiles": mixed,
        }
        for nm, w in [("wq", wq), ("wk", wk), ("wv", wv)]:
            wt = np.ascontiguousarray(w[sl, :].T)
            wh = _tf32(wt)
            m[nm + "Th"] = wh
            m[nm + "Tl"] = _tf32(wt - wh)
        in_maps.append(m)

    kwargs = {}
    if TRACE:
        kwargs = {"trace": True}
        if TRACE_DIR:
            kwargs["tmpdir"] = TRACE_DIR
    res = run_bass_kernel_spmd(nc, in_maps, core_ids=list(range(NCORES)), **kwargs)
    LAST_RESULT[0] = res

    acc = res.results[0]["out"].astype(np.float64)
    for c in range(1, NCORES):
        acc += res.results[c]["out"]
    return acc.astype(np.float32).reshape(B, S, D)


# revision 9
# speedup vs baseline: 1.0625x; 1.0544x over previous
"""Trainium2 Bass kernel for nn_AbstractAttention (dense transformer attention
with RoPE, B=2 S=2048 D=4096 H=32), tensor-parallel over heads on 8 cores.

Self-contained: builds the Bass program, shards inputs on host, runs via
run_bass_kernel_spmd, and reduces the partial outputs on host.

Numerics: all big matmuls use a 3-term tf32 (float32r) split
x@w ~= xh@wh + xh@wl + xl@wh (hi/lo split on host or on-device via DVE
rounding), which matches fp32 accuracy at ~3/4 the PE cost. Attention
probs@V stays native fp32. Softmax skips the max-subtraction (scores are
O(1) by construction), so exp/sum run in a single pass; the softmax
denominator rides in an extra all-ones column of V.
"""

import math

import numpy as np

import concourse.bacc as bacc
import concourse.mybir as mybir
from concourse.bass_utils import run_bass_kernel_spmd
from concourse.masks import make_identity
from concourse.tile import TileContext

P = 128  # partitions / head_dim
F32 = mybir.dt.float32
F32R = mybir.dt.float32r
ALU = mybir.AluOpType
ACTF = mybir.ActivationFunctionType

# set by test.py for profiling; grading path leaves these alone
TRACE = False
TRACE_DIR = None
LAST_RESULT = [None]

ZERO, MIXED, SKIP = 0, 1, 2


def _tf32(a):
    u = np.ascontiguousarray(a, dtype=np.float32).view(np.uint32)
    r = ((u >> 13).astype(np.uint64) + ((u >> 12) & 1)) << 13
    return (r & 0xFFFFFFFF).astype(np.uint32).view(np.float32)


def _classify_blocks(maskT, S, QT):
    """maskT: [S, S] (k, q). Returns (kinds[nqt][nkc], mixed_tiles, mixed_index)."""
    nqt, nkc = S // QT, S // P
    kinds = [[ZERO] * nkc for _ in range(nqt)]
    tiles = []
    index = {}
    for qt in range(nqt):
        for kc in range(nkc):
            sub = maskT[kc * P : (kc + 1) * P, qt * QT : (qt + 1) * QT]
            if np.all(sub == 0.0):
                kinds[qt][kc] = ZERO
            elif np.all(np.isneginf(sub) | (sub < -1e30)):
                kinds[qt][kc] = SKIP
            else:
                kinds[qt][kc] = MIXED
                index[(qt, kc)] = len(tiles)
                # pre-scale by sqrt(HD): kernel computes exp((S + m)/sqrt(HD))
                m = np.maximum(sub.astype(np.float64) * math.sqrt(P), -1e30)
                tiles.append(m.astype(np.float32))
    if tiles:
        mixed = np.ascontiguousarray(np.stack(tiles)).astype(np.float32)
    else:
        mixed = np.zeros((1, P, QT), dtype=np.float32)
    return kinds, mixed, index


def _build(B, S, D, HL, kinds, mixed_index, n_mixed):
    """Build the per-core Bass program (HL local heads, DL=HL*128 local dims)."""
    DL = HL * P
    T = B * S
    NCH = T // P
    CHB = S // P
    QT = 512
    NQT = S // QT
    NKC = S // P
    KD = D // P
    inv_sqrt_d = 1.0 / math.sqrt(P)

    nc = bacc.Bacc(None, target_bir_lowering=False)

    xTh = nc.declare_dram_parameter("xTh", [D, T], F32R, isOutput=False)
    xTl = nc.declare_dram_parameter("xTl", [D, T], F32R, isOutput=False)
    wqTh = nc.declare_dram_parameter("wqTh", [D, DL], F32R, isOutput=False)
    wqTl = nc.declare_dram_parameter("wqTl", [D, DL], F32R, isOutput=False)
    wkTh = nc.declare_dram_parameter("wkTh", [D, DL], F32R, isOutput=False)
    wkTl = nc.declare_dram_parameter("wkTl", [D, DL], F32R, isOutput=False)
    wvTh = nc.declare_dram_parameter("wvTh", [D, DL], F32R, isOutput=False)
    wvTl = nc.declare_dram_parameter("wvTl", [D, DL], F32R, isOutput=False)
    woTh = nc.declare_dram_parameter("woTh", [DL, D], F32R, isOutput=False)
    woTl = nc.declare_dram_parameter("woTl", [DL, D], F32R, isOutput=False)
    cos_e = nc.declare_dram_parameter("cos_e", [S, P], F32, isOutput=False)
    nsin_e = nc.declare_dram_parameter("nsin_e", [S, P], F32, isOutput=False)
    mtiles = nc.declare_dram_parameter(
        "mask_tiles", [max(n_mixed, 1), P, QT], F32, isOutput=False
    )
    out = nc.declare_dram_parameter("out", [T, D], F32, isOutput=True)

    ts = lambda i, s: slice(i * s, (i + 1) * s)

    with TileContext(nc) as tc:
        with (
            tc.tile_pool(name="consts", bufs=1) as consts,
            tc.tile_pool(name="stage", bufs=2) as stage,
            tc.tile_pool(name="small", bufs=4) as small,
            tc.tile_pool(name="dram", bufs=1, space="DRAM") as dram,
        ):
            ident = consts.tile([P, P], F32)
            make_identity(nc, ident)
            ones_h = consts.tile([P, HL], F32)
            nc.vector.memset(ones_h, 1.0)

            # DRAM scratch
            qT_scr = [
                [dram.tile([P, S], F32, tag=f"qT{b}_{h}", name=f"qT{b}_{h}")
                 for h in range(HL)]
                for b in range(B)
            ]
            kT_scr = [
                [dram.tile([P, S], F32, tag=f"kT{b}_{h}", name=f"kT{b}_{h}")
                 for h in range(HL)]
                for b in range(B)
            ]
            v_scr = [
                dram.tile([S, HL, P + 1], F32, tag=f"v{b}", name=f"v{b}")
                for b in range(B)
            ]
            cTh_scr = [
                dram.tile([DL, S], F32R, tag=f"cTh{b}", name=f"cTh{b}")
                for b in range(B)
            ]
            cTl_scr = [
                dram.tile([DL, S], F32R, tag=f"cTl{b}", name=f"cTl{b}")
                for b in range(B)
            ]

            for b in range(B):
                for c in range(CHB):
                    nc.sync.dma_start(
                        v_scr[b][ts(c, P), :, P : P + 1], ones_h[:, :, None]
                    )

            xTh_r = xTh.ap().rearrange("(o p) t -> p o t", p=P)
            xTl_r = xTl.ap().rearrange("(o p) t -> p o t", p=P)

            # ================= Phase 1: projections (+rope for q/k) ==========
            with (
                tc.tile_pool(name="p1c", bufs=1) as p1c,
                tc.tile_pool(name="wpool", bufs=1) as wpool,
                tc.tile_pool(name="xpool", bufs=2) as xpool,
                tc.tile_pool(name="work", bufs=2) as work,
                tc.tile_pool(name="psA1", bufs=2, space="PSUM") as psA1,
                tc.tile_pool(name="psT1", bufs=2, space="PSUM") as psT1,
            ):
                cos_sb = p1c.tile([P, CHB, P], F32)
                nsin_sb = p1c.tile([P, CHB, P], F32)
                nc.sync.dma_start(
                    cos_sb, cos_e.ap().rearrange("(o p) d -> p o d", p=P)
                )
                nc.sync.dma_start(
                    nsin_sb, nsin_e.ap().rearrange("(o p) d -> p o d", p=P)
                )
                pp = dram.tile([T, DL], F32, tag="pp", name="pp")

                def proj_pass(wTh, wTl, rope, name):
                    KH = KD // 2
                    for half in range(2):
                        w_h = wpool.tile([P, KH, DL], F32R, tag="wh")
                        nc.sync.dma_start(
                            w_h,
                            wTh.ap().rearrange("(o p) n -> p o n", p=P)[
                                :, ts(half, KH)
                            ],
                        )
                        w_l = wpool.tile([P, KH, DL], F32R, tag="wl")
                        nc.sync.dma_start(
                            w_l,
                            wTl.ap().rearrange("(o p) n -> p o n", p=P)[
                                :, ts(half, KH)
                            ],
                        )
                        for ch in range(NCH):
                            b, so = ch // CHB, ch % CHB
                            x_h = xpool.tile([P, KH, P], F32R, tag="xh")
                            nc.sync.dma_start(
                                x_h, xTh_r[:, ts(half, KH), ts(ch, P)]
                            )
                            x_l = xpool.tile([P, KH, P], F32R, tag="xl")
                            nc.sync.dma_start(
                                x_l, xTl_r[:, ts(half, KH), ts(ch, P)]
                            )
                            ps = psA1.tile([P, DL], F32, tag="ps_a")
                            i = 0
                            nterm = 3 * KH
                            for a, w in [(x_h, w_h), (x_h, w_l), (x_l, w_h)]:
                                for j in range(KH):
                                    nc.tensor.matmul(
                                        ps,
                                        a[:, j, :],
                                        w[:, j, :],
                                        start=(i == 0),
                                        stop=(i == nterm - 1),
                                    )
                                    i += 1
                            if half == 0:
                                pst = stage.tile([P, DL], F32, tag="pst")
                                nc.scalar.copy(pst, ps)
                                nc.sync.dma_start(pp[ts(ch, P), :], pst)
                                continue
                            pa = stage.tile([P, DL], F32, tag="pa")
                            nc.sync.dma_start(pa, pp[ts(ch, P), :])
                            if not rope:
                                vst = stage.tile([P, DL], F32, tag="vst")
                                nc.vector.tensor_tensor(vst, ps, pa, ALU.add)
                                nc.sync.dma_start(
                                    v_scr[b][ts(so, P), :, 0:P],
                                    vst.rearrange("p (h d) -> p h d", h=HL),
                                )
                                continue
                            qs = work.tile([P, DL], F32, tag="qs")
                            nc.vector.tensor_tensor(qs, ps, pa, ALU.add)
                            cosv = cos_sb[:, so, None, :].to_broadcast((P, HL, P))
                            t1 = work.tile([P, DL], F32, tag="t1")
                            t1v = t1.rearrange("p (h d) -> p h d", h=HL)
                            qsv = qs.rearrange("p (h d) -> p h d", h=HL)
                            nc.vector.tensor_tensor(t1v, qsv, cosv, ALU.mult)
                            t2 = work.tile([P, DL], F32, tag="t2")
                            qs4 = qs.rearrange(
                                "p (h n two) -> p h n two", h=HL, two=2
                            )
                            t24 = t2.rearrange(
                                "p (h n two) -> p h n two", h=HL, two=2
                            )
                            nsv = nsin_sb[:, so, :].rearrange(
                                "p (n two) -> p n two", two=2
                            )
                            nc.vector.tensor_tensor(
                                t24[:, :, :, 0:1],
                                qs4[:, :, :, 1:2],
                                nsv[:, None, :, 0:1].to_broadcast(
                                    (P, HL, P // 2, 1)
                                ),
                                ALU.mult,
                            )
                            nc.vector.tensor_tensor(
                                t24[:, :, :, 1:2],
                                qs4[:, :, :, 0:1],
                                nsv[:, None, :, 1:2].to_broadcast(
                                    (P, HL, P // 2, 1)
                                ),
                                ALU.mult,
                            )
                            nc.vector.tensor_tensor(t1, t1, t2, ALU.add)
                            t1v = t1.rearrange("p (h d) -> p h d", h=HL)
                            scr = qT_scr if name == "q" else kT_scr
                            for h in range(HL):
                                pt = psT1.tile([P, P], F32, tag="ptr")
                                nc.tensor.transpose(pt, t1v[:, h, :], ident)
                                st = stage.tile([P, P], F32, tag="trs")
                                nc.scalar.copy(st, pt)
                                nc.sync.dma_start(scr[b][h][:, ts(so, P)], st)

                proj_pass(wqTh, wqTl, True, "q")
                proj_pass(wkTh, wkTl, True, "k")
                proj_pass(wvTh, wvTl, False, "v")

            # ================= Phase 2: attention ============================
            active = [
                [kc for kc in range(NKC) if kinds[qt][kc] != SKIP]
                for qt in range(NQT)
            ]
            with (
                tc.tile_pool(name="mpool", bufs=1) as mpool,
                tc.tile_pool(name="a2", bufs=1) as a2,
                tc.tile_pool(name="qtp", bufs=2) as qtp,
                tc.tile_pool(name="work2", bufs=3) as work2,
                tc.tile_pool(name="psA2", bufs=3, space="PSUM") as psA2,
                tc.tile_pool(name="psCtx", bufs=1, space="PSUM") as psCtx,
                tc.tile_pool(name="psT2", bufs=1, space="PSUM") as psT2,
            ):
                mcache = {}
                if n_mixed <= 20:
                    for (qt, kc), idx in mixed_index.items():
                        mt = mpool.tile(
                            [P, QT], F32, tag=f"m{qt}_{kc}", name=f"m{qt}_{kc}"
                        )
                        nc.sync.dma_start(mt, mtiles.ap()[idx])
                        mcache[(qt, kc)] = mt

                for b in range(B):
                    for h in range(HL):
                        kt_t = a2.tile([P, S], F32, tag="kt_t")
                        nc.sync.dma_start(kt_t, kT_scr[b][h][:])
                        kth = a2.tile([P, S], F32R, tag="kth")
                        nc.vector.tensor_copy(kth, kt_t)
                        ktl = a2.tile([P, S], F32R, tag="ktl")
                        nc.vector.tensor_tensor(ktl, kt_t, kth, ALU.subtract)
                        v_sb = a2.tile([P, NKC, P + 1], F32, tag="v_sb")
                        nc.sync.dma_start(
                            v_sb,
                            v_scr[b][:, h, :].rearrange("(o p) c -> p o c", p=P),
                        )
                        for qt in range(NQT):
                            acts = active[qt]
                            if not acts:
                                continue
                            qt_t = qtp.tile([P, QT], F32, tag="qt_t")
                            nc.sync.dma_start(
                                qt_t, qT_scr[b][h][:, ts(qt, QT)]
                            )
                            qth = qtp.tile([P, QT], F32R, tag="qth")
                            nc.vector.tensor_copy(qth, qt_t)
                            qtl = qtp.tile([P, QT], F32R, tag="qtl")
                            nc.vector.tensor_tensor(qtl, qt_t, qth, ALU.subtract)
                            ctx_ps = [
                                psCtx.tile(
                                    [P, P + 1], F32, tag=f"ctx{i}", name=f"ctx{i}"
                                )
                                for i in range(QT // P)
                            ]
                            for ki, kc in enumerate(acts):
                                sps = psA2.tile([P, QT], F32, tag="ps_a2")
                                for ti, (lh, rh) in enumerate(
                                    [(kth, qth), (kth, qtl), (ktl, qth)]
                                ):
                                    nc.tensor.matmul(
                                        sps,
                                        lh[:, ts(kc, P)],
                                        rh,
                                        start=(ti == 0),
                                        stop=(ti == 2),
                                    )
                                pT = work2.tile([P, QT], F32, tag="pT")
                                if kinds[qt][kc] == MIXED:
                                    if (qt, kc) in mcache:
                                        mt = mcache[(qt, kc)]
                                    else:
                                        mt = work2.tile([P, QT], F32, tag="mload")
                                        nc.sync.dma_start(
                                            mt, mtiles.ap()[mixed_index[(qt, kc)]]
                                        )
                                    nc.vector.tensor_tensor(sps, sps, mt, ALU.add)
                                nc.scalar.activation(
                                    pT, sps, ACTF.Exp, scale=inv_sqrt_d
                                )
                                for qsi in range(QT // P):
                                    nc.tensor.matmul(
                                        ctx_ps[qsi],
                                        pT[:, ts(qsi, P)],
                                        v_sb[:, kc, :],
                                        start=(ki == 0),
                                        stop=(ki == len(acts) - 1),
                                    )
                            for qsi in range(QT // P):
                                rec = small.tile([P, 1], F32, tag="rec")
                                nc.vector.reciprocal(rec, ctx_ps[qsi][:, P : P + 1])
                                cn = stage.tile([P, P], F32, tag="cn")
                                nc.vector.tensor_scalar_mul(
                                    cn, ctx_ps[qsi][:, 0:P], rec
                                )
                                pt = psT2.tile([P, P], F32, tag="ptr2")
                                nc.tensor.transpose(pt, cn, ident)
                                chi = stage.tile([P, P], F32R, tag="chi")
                                nc.scalar.copy(chi, pt)
                                clo = stage.tile([P, P], F32R, tag="clo")
                                nc.vector.tensor_tensor(clo, pt, chi, ALU.subtract)
                                qcol = ts(qt * (QT // P) + qsi, P)
                                nc.sync.dma_start(cTh_scr[b][ts(h, P), qcol], chi)
                                nc.sync.dma_start(cTl_scr[b][ts(h, P), qcol], clo)

            # ================= Phase 3: output projection ====================
            with (
                tc.tile_pool(name="wop", bufs=1) as wop,
                tc.tile_pool(name="cpool", bufs=2) as cpool,
                tc.tile_pool(name="psA3", bufs=3, space="PSUM") as psA3,
            ):
                wo_h = wop.tile([P, HL, D], F32R, tag="woh")
                nc.sync.dma_start(
                    wo_h, woTh.ap().rearrange("(o p) n -> p o n", p=P)
                )
                wo_l = wop.tile([P, HL, D], F32R, tag="wol")
                nc.sync.dma_start(
                    wo_l, woTl.ap().rearrange("(o p) n -> p o n", p=P)
                )
                for b in range(B):
                    ch_r = cTh_scr[b].rearrange("(o p) t -> p o t", p=P)
                    cl_r = cTl_scr[b].rearrange("(o p) t -> p o t", p=P)
                    for c in range(CHB):
                        cTh = cpool.tile([P, HL, P], F32R, tag="cth3")
                        nc.sync.dma_start(cTh, ch_r[:, :, ts(c, P)])
                        cTl = cpool.tile([P, HL, P], F32R, tag="ctl3")
                        nc.sync.dma_start(cTl, cl_r[:, :, ts(c, P)])
                        for oc in range(D // 512):
                            ps = psA3.tile([P, 512], F32, tag="ps_a3")
                            i = 0
                            for lh, rh in [(cTh, wo_h), (cTh, wo_l), (cTl, wo_h)]:
                                for j in range(HL):
                                    nc.tensor.matmul(
                                        ps,
                                        lh[:, j, :],
                                        rh[:, j, ts(oc, 512)],
                                        start=(i == 0),
                                        stop=(i == 3 * HL - 1),
                                    )
                                    i += 1
                            st = stage.tile([P, 512], F32, tag="ost")
                            nc.scalar.copy(st, ps)
                            nc.sync.dma_start(
                                out.ap()[ts(b * CHB + c, P), ts(oc, 512)], st
                            )

    nc.finalize()
    return nc


def kernel(x, wq, wk, wv, wo, cos, sin, mask):
    B, S, D = x.shape
    H = D // P
    NCORES = 8
    HL = H // NCORES
    DL = HL * P
    T = B * S

    x = np.asarray(x, dtype=np.float32)
    xT = np.ascontiguousarray(x.reshape(T, D).T)
    xTh = _tf32(xT)
    xTl = _tf32(xT - xTh)
    cos = np.asarray(cos, dtype=np.float32)
    sin = np.asarray(sin, dtype=np.float32)
    cos_e = np.repeat(cos, 2, axis=1).astype(np.float32)  # [S, 128]
    nsin_e = np.empty((S, P), dtype=np.float32)
    nsin_e[:, 0::2] = -sin
    nsin_e[:, 1::2] = sin

    maskT = np.ascontiguousarray(np.asarray(mask, dtype=np.float32)[0, 0].T)
    kinds, mixed, mixed_index = _classify_blocks(maskT, S, 512)

    nc = _build(B, S, D, HL, kinds, mixed_index, len(mixed))

    wq = np.asarray(wq, dtype=np.float32)
    wk = np.asarray(wk, dtype=np.float32)
    wv = np.asarray(wv, dtype=np.float32)
    wo = np.asarray(wo, dtype=np.float32)

    in_maps = []
    for c in range(NCORES):
        sl = slice(c * DL, (c + 1) * DL)
        m = {
            "xTh": xTh,
            "xTl": xTl,
            "cos_e": cos_e,
            "nsin_e": nsin_e,
            "mask_tiles": mixed,
        }
        for nm, w in [("wq", wq), ("wk", wk), ("wv", wv)]:
            wt = np.ascontiguousarray(w[sl, :].T)
            wh = _tf32(wt)
            m[nm + "Th"] = wh
            m[nm + "Tl"] = _tf32(wt - wh)
        wot = np.ascontiguousarray(wo[:, sl].T)
        m["woTh"] = _tf32(wot)
        m["woTl"] = _tf32(wot - m["woTh"])
        in_maps.append(m)

    kwargs = {}
    if TRACE:
        kwargs = {"trace": True}
        if TRACE_DIR:
            kwargs["tmpdir"] = TRACE_DIR
    res = run_bass_kernel_spmd(nc, in_maps, core_ids=list(range(NCORES)), **kwargs)
    LAST_RESULT[0] = res

    acc = res.results[0]["out"].astype(np.float64)
    for c in range(1, NCORES):
        acc += res.results[c]["out"]
    return acc.astype(np.float32).reshape(B, S, D)
